# revision 19
# baseline (speedup 1.0000x reference)
"""PoseVQVAE forward — fp8(e4m3) DoubleRow encoder + VQ scoring, fp16 decoder.

Same structure as the fp16 kernel, but the encoder (L1..L3, Wmu) and the VQ
scoring matmul run in fp8-e4m3 with DoubleRow (2 contraction rows/cell, 2x
PE throughput). Power-of-2 scales keep every tensor comfortably inside e4m3
range (TRN e4m3 overflows to Inf at 256); fp32 PSUM accumulation. Validated
offline: VQ argmin decision margins are >=3.5 vs ~0.1 worst-case score
error, so the selected codebook rows — and therefore the decoder input and
recon — are unchanged vs the fp32 pipeline. Decoder stays fp16; c ships
separately in fp16 for it.
"""
import sys

sys.path.insert(0, "/opt/trn_rl_repo")

import ml_dtypes
import numpy as np

import concourse.bass as bass
import concourse.bacc as bacc
import concourse.mybir as mybir
import concourse.tile as tile
from concourse.bass_utils import run_bass_kernel_spmd
from concourse.masks import make_identity

F32 = mybir.dt.float32
F16 = mybir.dt.float16
F8 = mybir.dt.float8e4
U32 = mybir.dt.uint32
AF = mybir.ActivationFunctionType
DR = mybir.MatmulPerfMode.DoubleRow

B = 131072
NCORES = 8
BS = B // NCORES
FRAME = 267
SIN = 2 * FRAME
SINP = 768  # 3 x 256 DoubleRow pair chunks
CP = 384  # c rows padded to 3 x 128 for the decoder
H1 = 512
LAT = 256
K = 1024
BT = 512
NG = BT // 128

S_S, S_W, S_H1, S_H, S_MU, S_E = 16.0, 64.0, 32.0, 64.0, 256.0, 32.0


def _kchunks(n):
    out = []
    r = 0
    while r < n:
        out.append((r, min(128, n - r)))
        r += 128
    return out


def build_nc(bs=BS):
    nt = bs // BT
    nc = bacc.Bacc("TRN2", target_bir_lowering=False)

    sT8 = nc.dram_tensor("sT8", [SINP // 2, 2 * bs], F8, kind="ExternalInput")
    cT = nc.dram_tensor("cT", [CP, bs], F16, kind="ExternalInput")
    w_in = {}
    for name, shp, dt in (
        ("w1", [SINP // 2, 2 * H1], F8), ("w2", [H1 // 2, 2 * H1], F8),
        ("w3", [H1 // 2, 2 * H1], F8), ("wmu", [H1 // 2, 2 * LAT], F8),
        ("w4q", [LAT, H1], F16), ("w4c", [CP, H1], F16),
        ("w5", [H1, H1], F16), ("w6", [H1, H1], F16), ("wo", [H1, FRAME], F16),
    ):
        w_in[name] = nc.dram_tensor(name, shp, dt, kind="ExternalInput")
    b_in = {}
    for name, cols in (("b1", 4), ("b2", 4), ("b3", 4), ("bmu", 2),
                       ("bmusq", 2), ("b4", 4), ("b5", 4), ("b6", 4),
                       ("bo", 3)):
        b_in[name] = nc.dram_tensor(name, [128, cols], F32, kind="ExternalInput")
    embp = nc.dram_tensor("embp", [LAT // 2, 2 * K], F8, kind="ExternalInput")
    e2bc = nc.dram_tensor("e2bc", [128, K], F32, kind="ExternalInput")
    embT = nc.dram_tensor("embT", [K, LAT], F16, kind="ExternalInput")

    reconT = nc.dram_tensor("reconT", [FRAME, bs], F32, kind="ExternalOutput")
    inds_o = nc.dram_tensor("inds", [128, nt * NG], U32, kind="ExternalOutput")
    maxs_o = nc.dram_tensor("maxs", [128, nt * NG], F32, kind="ExternalOutput")
    musq_o = nc.dram_tensor("musq", [128, nt * 2], F32, kind="ExternalOutput")

    def pair(ap2d):
        return ap2d.rearrange("p (two n) -> p two n", two=2)

    with tile.TileContext(nc) as tc:
        with tc.tile_pool(name="consts", bufs=1) as cp_, \
             tc.tile_pool(name="io", bufs=3) as io, \
             tc.tile_pool(name="act", bufs=2) as ap_, \
             tc.tile_pool(name="ps", bufs=4, space="PSUM") as ps, \
             tc.tile_pool(name="psS", bufs=2, space="PSUM") as psS, \
             tc.tile_pool(name="psT", bufs=2, space="PSUM") as psT:

            wsb = {}
            for name in ("w1", "w2", "w3", "wmu", "w4q", "w4c", "w5", "w6",
                         "wo"):
                rows, cols = w_in[name].shape
                dt = w_in[name].dtype
                chunks = []
                for ci, (r0, rr) in enumerate(_kchunks(rows)):
                    t = cp_.tile([128, cols], dt, tag=f"{name}_{ci}")
                    nc.sync.dma_start(out=t[:rr], in_=w_in[name][r0:r0 + rr, :])
                    chunks.append(t)
                wsb[name] = chunks
            wsb["w4"] = [wsb["w4q"][0], wsb["w4q"][1],
                         wsb["w4c"][0], wsb["w4c"][1], wsb["w4c"][2]]
            bsb = {}
            for name in b_in:
                t = cp_.tile(list(b_in[name].shape), F32, tag=f"bs_{name}")
                nc.sync.dma_start(out=t[:], in_=b_in[name][:])
                bsb[name] = t
            emb_sb = cp_.tile([128, 2 * K], F8, tag="embp_sb")
            nc.sync.dma_start(out=emb_sb[:], in_=embp[:])
            e2_sb = cp_.tile([128, K], F32, tag="e2sb")
            nc.sync.dma_start(out=e2_sb[:], in_=e2bc[:])
            ident = cp_.tile([128, 128], F16, tag="ident")
            make_identity(nc, ident[:])

            inds_sb = cp_.tile([128, nt * NG], U32, tag="inds_sb")
            maxs_sb = cp_.tile([128, nt * NG], F32, tag="maxs_sb")
            musq_sb = cp_.tile([128, nt * 2], F32, tag="musq_sb")

            def dlayer(hpairs, wname, m_out, scale, bname, tprefix,
                       out_pairs=True, extra_epilogue=None):
                w_chunks = wsb[wname]
                nk = len(hpairs)
                outs = [ap_.tile([128, 2 * BT], F8, tag=f"{tprefix}_{i}",
                                 name=f"{tprefix}_{i}")
                        for i in range(m_out // 256)] if out_pairs else []
                for m, (mr0, mrows) in enumerate(_kchunks(m_out)):
                    p = ps.tile([128, BT], F32, tag="mm")
                    for ci in range(nk):
                        nc.tensor.matmul(
                            p[:mrows],
                            pair(w_chunks[ci][:])[:, :, mr0:mr0 + mrows],
                            pair(hpairs[ci][:]),
                            start=(ci == 0), stop=(ci == nk - 1),
                            perf_mode=DR,
                        )
                    if out_pairs:
                        o = outs[m // 2][:, (m % 2) * BT:(m % 2 + 1) * BT]
                        nc.scalar.activation(o, p[:mrows], AF.Relu,
                                             scale=scale,
                                             bias=bsb[bname][:mrows, m:m + 1])
                    if extra_epilogue is not None:
                        extra_epilogue(m, mrows, p)
                return outs

            def layer(h_chunks, h_ks, wname, bname, m_out, func, out_dt,
                      tprefix):
                outs = []
                w_chunks = wsb[wname]
                for m, (mr0, mrows) in enumerate(_kchunks(m_out)):
                    p = ps.tile([128, BT], F32, tag="mm")
                    nk = len(h_chunks)
                    for ki2 in range(nk):
                        nc.tensor.matmul(
                            p[:mrows],
                            w_chunks[ki2][:h_ks[ki2], mr0:mr0 + mrows],
                            h_chunks[ki2][:h_ks[ki2]],
                            start=(ki2 == 0), stop=(ki2 == nk - 1),
                        )
                    o = ap_.tile([128, BT], out_dt, tag=f"{tprefix}_{m}")
                    nc.scalar.activation(o[:mrows], p[:mrows], func,
                                         bias=bsb[bname][:mrows, m:m + 1])
                    outs.append(o)
                return outs

            hk = [128] * 4
            pend = None

            def decode(q_nats, cTt, t):
                b0 = t * BT
                qT = [ap_.tile([128, BT], F16, tag=f"qT_{lc}",
                               name=f"qT_{lc}_{t}") for lc in range(2)]
                for g in range(NG):
                    gsl = slice(g * 128, (g + 1) * 128)
                    for lc in range(2):
                        pT = psT.tile([128, 128], F16, tag="pT")
                        nc.tensor.transpose(
                            pT[:], q_nats[g][:, lc * 128:(lc + 1) * 128],
                            ident[:])
                        if lc == 0:
                            nc.scalar.activation(qT[lc][:, gsl], pT[:],
                                                 AF.Copy)
                        else:
                            nc.vector.tensor_copy(qT[lc][:, gsl], pT[:])
                d_chunks = [qT[0], qT[1], cTt[0], cTt[1], cTt[2]]
                hd = layer(d_chunks, [128] * 5, "w4", "b4", H1, AF.Relu,
                           F16, "hd1")
                hd = layer(hd, hk, "w5", "b5", H1, AF.Relu, F16, "hd2")
                hd = layer(hd, hk, "w6", "b6", H1, AF.Relu, F16, "hd3")
                ro = layer(hd, hk, "wo", "bo", FRAME, AF.Identity, F32, "ro")
                for m, (mr0, mrows) in enumerate(_kchunks(FRAME)):
                    nc.sync.dma_start(out=reconT[mr0:mr0 + mrows, b0:b0 + BT],
                                      in_=ro[m][:mrows])

            for t in range(nt):
                b0 = t * BT
                sTt = []
                for ci in range(3):
                    tt = io.tile([128, 2 * BT], F8, tag=f"sT_{ci}",
                                 name=f"sT_{ci}_{t}")
                    nc.sync.dma_start(
                        out=tt[:, 0:BT],
                        in_=sT8[ci * 128:(ci + 1) * 128, b0:b0 + BT])
                    nc.sync.dma_start(
                        out=tt[:, BT:2 * BT],
                        in_=sT8[ci * 128:(ci + 1) * 128, bs + b0:bs + b0 + BT])
                    sTt.append(tt)
                cTt = []
                for ci in range(3):
                    tt = io.tile([128, BT], F16, tag=f"cT_{ci}",
                                 name=f"cT_{ci}_{t}")
                    nc.sync.dma_start(out=tt[:],
                                      in_=cT[ci * 128:(ci + 1) * 128,
                                             b0:b0 + BT])
                    cTt.append(tt)

                # encoder: L1 psum=1024*pre -> h1=32*relu; L2 psum=2048*pre
                # -> h2=64*relu; L3 psum=4096*pre -> h3=64*relu
                h = dlayer(sTt, "w1", H1, 1.0 / 32.0, "b1", "h1")
                h = dlayer(h, "w2", H1, 1.0 / 32.0, "b2", "h2")
                h = dlayer(h, "w3", H1, 1.0 / 64.0, "b3", "h3")

                mu8 = ap_.tile([128, 2 * BT], F8, tag="mu8", name=f"mu8_{t}")

                def mu_epi(m, mrows, p, t=t, mu8=mu8):
                    nc.scalar.activation(
                        mu8[:, m * BT:(m + 1) * BT], p[:mrows], AF.Identity,
                        scale=1.0 / 16.0, bias=bsb["bmu"][:mrows, m:m + 1])
                    scr = ap_.tile([128, BT], F32, tag="musq_scr")
                    nc.scalar.activation(
                        scr[:mrows], p[:mrows], AF.Square,
                        scale=1.0 / 4096.0,
                        bias=bsb["bmusq"][:mrows, m:m + 1],
                        accum_out=musq_sb[:mrows, 2 * t + m:2 * t + m + 1])

                dlayer(h, "wmu", LAT, None, None, "muu", out_pairs=False,
                       extra_epilogue=mu_epi)

                q_nats = []
                for g in range(NG):
                    col = t * NG + g
                    S_sb = io.tile([128, K], F32, tag="S_sb")
                    for half in range(2):
                        hsl = slice(half * 512, (half + 1) * 512)
                        pS = psS.tile([128, 512], F32, tag="S")
                        nc.tensor.matmul(
                            pS[:],
                            pair(mu8[:])[:, :, g * 128:(g + 1) * 128],
                            pair(emb_sb[:])[:, :, hsl],
                            start=True, stop=True, perf_mode=DR)
                        nc.vector.tensor_tensor(out=S_sb[:, hsl], in0=pS[:],
                                                in1=e2_sb[:, hsl],
                                                op=mybir.AluOpType.add)
                    mv = io.tile([128, 8], F32, tag="mv")
                    mi = io.tile([128, 8], U32, tag="mi")
                    nc.vector.max(mv[:], S_sb[:])
                    nc.vector.max_index(mi[:], mv[:], S_sb[:])
                    nc.vector.tensor_copy(inds_sb[:, col:col + 1], mi[:, 0:1])
                    nc.vector.tensor_copy(maxs_sb[:, col:col + 1], mv[:, 0:1])

                    q_nat = io.tile([128, LAT], F16, tag=f"q_nat_{g}",
                                    name=f"q_nat_{g}_{t}", bufs=2)
                    nc.gpsimd.indirect_dma_start(
                        out=q_nat[:], out_offset=None, in_=embT[:],
                        in_offset=bass.IndirectOffsetOnAxis(
                            ap=inds_sb[:, col:col + 1], axis=0),
                    )
                    q_nats.append(q_nat)

                if pend is not None:
                    decode(*pend)
                pend = (q_nats, cTt, t)

            decode(*pend)

            nc.sync.dma_start(out=inds_o[:], in_=inds_sb[:])
            nc.sync.dma_start(out=maxs_o[:], in_=maxs_sb[:])
            nc.sync.dma_start(out=musq_o[:], in_=musq_sb[:])
    nc.compile()
    return nc


def make_in_maps(x, c, W1, b1, W2, b2, W3, b3, Wmu, bmu,
                 W4, b4, W5, b5, W6, b6, Wo, bo, embed, bs=BS, ncores=NCORES):
    f16 = np.float16
    f32 = np.float32
    f8 = ml_dtypes.float8_e4m3

    def bias_cols(b, ncols, scale=1.0):
        out = np.zeros((128, ncols), f32)
        b = np.asarray(b, f32) * scale
        for ci in range(ncols):
            seg = b[128 * ci:128 * (ci + 1)]
            out[:len(seg), ci] = seg
        return out

    def pair8(w, rows_pad, scale):
        wp = np.zeros((rows_pad, w.shape[1]), f32)
        wp[:w.shape[0]] = np.asarray(w, f32) * scale
        npair = rows_pad // 256
        v = wp.reshape(npair, 2, 128, w.shape[1])
        v = v.transpose(0, 2, 1, 3).reshape(npair * 128, 2 * w.shape[1])
        return np.ascontiguousarray(v.astype(f8))

    embed = np.asarray(embed, f32)
    e2 = np.sum(embed.astype(np.float64) ** 2, axis=0).astype(f32)
    w4 = np.asarray(W4, f32)
    w4c = np.zeros((CP, H1), f32)
    w4c[:SIN - FRAME, :] = w4[LAT:, :]

    common = dict(
        w1=pair8(W1, SINP, S_W), w2=pair8(W2, H1, S_W), w3=pair8(W3, H1, S_W),
        wmu=pair8(Wmu, H1, S_W),
        w4q=w4[:LAT, :].astype(f16), w4c=w4c.astype(f16),
        w5=np.asarray(W5, f32).astype(f16), w6=np.asarray(W6, f32).astype(f16),
        wo=np.asarray(Wo, f32).astype(f16),
        b1=bias_cols(b1, 4, S_H1), b2=bias_cols(b2, 4, S_H),
        b3=bias_cols(b3, 4, S_H), bmu=bias_cols(bmu, 2, S_MU),
        bmusq=bias_cols(bmu, 2, 1.0),
        b4=bias_cols(b4, 4), b5=bias_cols(b5, 4), b6=bias_cols(b6, 4),
        bo=bias_cols(bo, 3),
        embp=pair8(embed, LAT, S_E),
        e2bc=np.ascontiguousarray(
            np.broadcast_to((-0.5 * S_MU * S_E * e2)[None, :], (128, K))),
        embT=np.ascontiguousarray(embed.T.astype(f16)),
    )
    x = np.asarray(x, f32)
    c = np.asarray(c, f32)
    in_maps = []
    for i in range(ncores):
        sl = slice(i * bs, (i + 1) * bs)
        sp = np.zeros((SINP, bs), f32)
        sp[:FRAME] = x[sl].T
        sp[FRAME:SIN] = c[sl].T
        v = (sp * S_S).reshape(3, 2, 128, bs).transpose(0, 2, 1, 3)
        m = dict(common)
        m["sT8"] = np.ascontiguousarray(
            v.reshape(SINP // 2, 2 * bs).astype(f8))
        cpd = np.zeros((CP, bs), f16)
        cpd[:FRAME] = c[sl].T.astype(f16)
        m["cT"] = cpd
        in_maps.append(m)
    return in_maps


def postprocess(results, bs=BS):
    recon_parts = []
    musq_total = 0.0
    maxs_total = 0.0
    counts = np.zeros(K, np.int64)
    for r in results:
        recon_parts.append(np.ascontiguousarray(r["reconT"].T))
        musq_total += r["musq"].astype(np.float64).sum()
        maxs_total += r["maxs"].astype(np.float64).sum() / (S_MU * S_E)
        idx = r["inds"].T.reshape(-1).astype(np.int64)
        counts += np.bincount(idx, minlength=K)
    recon = np.concatenate(recon_parts, axis=0).astype(np.float32)
    n = counts.sum()
    loss = np.float32((musq_total - 2.0 * maxs_total) / (n * LAT))
    p = (counts / n).astype(np.float32)
    perp = np.float32(np.exp(-np.sum(p * np.log(p + np.float32(1e-10)))))
    return recon, loss, perp


_NC_CACHE = {}


def kernel(x, c, W1, b1, W2, b2, W3, b3, Wmu, bmu,
           W4, b4, W5, b5, W6, b6, Wo, bo, embed, **run_kwargs):
    if BS not in _NC_CACHE:
        _NC_CACHE[BS] = build_nc(BS)
    nc = _NC_CACHE[BS]
    in_maps = make_in_maps(x, c, W1, b1, W2, b2, W3, b3, Wmu, bmu,
                           W4, b4, W5, b5, W6, b6, Wo, bo, embed)
    res = run_bass_kernel_spmd(nc, in_maps, list(range(NCORES)), **run_kwargs)
    out = postprocess(res.results)
    kernel.last_results = res
    return out


# revision 20
# speedup vs baseline: 1.0137x; 1.0137x over previous
"""PoseVQVAE forward — fp8(e4m3) DoubleRow encoder + VQ scoring, fp16 decoder.

Same structure as the fp16 kernel, but the encoder (L1..L3, Wmu) and the VQ
scoring matmul run in fp8-e4m3 with DoubleRow (2 contraction rows/cell, 2x
PE throughput). Power-of-2 scales keep every tensor comfortably inside e4m3
range (TRN e4m3 overflows to Inf at 256); fp32 PSUM accumulation. Validated
offline: VQ argmin decision margins are >=3.5 vs ~0.1 worst-case score
error, so the selected codebook rows — and therefore the decoder input and
recon — are unchanged vs the fp32 pipeline. Decoder stays fp16; c ships
separately in fp16 for it.
"""
import sys

sys.path.insert(0, "/opt/trn_rl_repo")

import ml_dtypes
import numpy as np

import concourse.bass as bass
import concourse.bacc as bacc
import concourse.mybir as mybir
import concourse.tile as tile
from concourse.bass_utils import run_bass_kernel_spmd
from concourse.masks import make_identity

F32 = mybir.dt.float32
F16 = mybir.dt.float16
F8 = mybir.dt.float8e4
U32 = mybir.dt.uint32
AF = mybir.ActivationFunctionType
DR = mybir.MatmulPerfMode.DoubleRow

B = 131072
NCORES = 8
BS = B // NCORES
FRAME = 267
SIN = 2 * FRAME
SINP = 768  # 3 x 256 DoubleRow pair chunks
CP = 384  # c rows padded to 3 x 128 for the decoder
H1 = 512
LAT = 256
K = 1024
BT = 512
NG = BT // 128

S_S, S_W, S_H1, S_H, S_MU, S_E = 16.0, 64.0, 32.0, 64.0, 256.0, 32.0


def _kchunks(n):
    out = []
    r = 0
    while r < n:
        out.append((r, min(128, n - r)))
        r += 128
    return out


def build_nc(bs=BS):
    nt = bs // BT
    nc = bacc.Bacc("TRN2", target_bir_lowering=False)

    sT8 = nc.dram_tensor("sT8", [SINP // 2, 2 * bs], F8, kind="ExternalInput")
    cT = nc.dram_tensor("cT", [CP, bs], F16, kind="ExternalInput")
    w_in = {}
    for name, shp, dt in (
        ("w1", [SINP // 2, 2 * H1], F8), ("w2", [H1 // 2, 2 * H1], F8),
        ("w3", [H1 // 2, 2 * H1], F8), ("wmu", [H1 // 2, 2 * LAT], F8),
        ("w4q", [LAT, H1], F16), ("w4c", [CP, H1], F16),
        ("w5", [H1, H1], F16), ("w6", [H1, H1], F16), ("wo", [H1, FRAME], F16),
    ):
        w_in[name] = nc.dram_tensor(name, shp, dt, kind="ExternalInput")
    b_in = {}
    for name, cols in (("b1", 4), ("b2", 4), ("b3", 4), ("bmu", 2),
                       ("bmusq", 2), ("b4", 4), ("b5", 4), ("b6", 4),
                       ("bo", 3)):
        b_in[name] = nc.dram_tensor(name, [128, cols], F32, kind="ExternalInput")
    embp = nc.dram_tensor("embp", [LAT // 2, 2 * K], F8, kind="ExternalInput")
    e2bc = nc.dram_tensor("e2bc", [128, K], F32, kind="ExternalInput")
    embT = nc.dram_tensor("embT", [K, LAT], F16, kind="ExternalInput")

    reconT = nc.dram_tensor("reconT", [FRAME, bs], F32, kind="ExternalOutput")
    inds_o = nc.dram_tensor("inds", [128, nt * NG], U32, kind="ExternalOutput")
    maxs_o = nc.dram_tensor("maxs", [128, nt * NG], F32, kind="ExternalOutput")
    musq_o = nc.dram_tensor("musq", [128, nt * 2], F32, kind="ExternalOutput")

    def pair(ap2d):
        return ap2d.rearrange("p (two n) -> p two n", two=2)

    with tile.TileContext(nc) as tc:
        with tc.tile_pool(name="consts", bufs=1) as cp_, \
             tc.tile_pool(name="io", bufs=3) as io, \
             tc.tile_pool(name="act", bufs=2) as ap_, \
             tc.tile_pool(name="ps", bufs=4, space="PSUM") as ps, \
             tc.tile_pool(name="psS", bufs=2, space="PSUM") as psS, \
             tc.tile_pool(name="psT", bufs=2, space="PSUM") as psT:

            wsb = {}
            for name in ("w1", "w2", "w3", "wmu", "w4q", "w4c", "w5", "w6",
                         "wo"):
                rows, cols = w_in[name].shape
                dt = w_in[name].dtype
                chunks = []
                for ci, (r0, rr) in enumerate(_kchunks(rows)):
                    t = cp_.tile([128, cols], dt, tag=f"{name}_{ci}")
                    nc.sync.dma_start(out=t[:rr], in_=w_in[name][r0:r0 + rr, :])
                    chunks.append(t)
                wsb[name] = chunks
            wsb["w4"] = [wsb["w4q"][0], wsb["w4q"][1],
                         wsb["w4c"][0], wsb["w4c"][1], wsb["w4c"][2]]
            bsb = {}
            for name in b_in:
                t = cp_.tile(list(b_in[name].shape), F32, tag=f"bs_{name}")
                nc.sync.dma_start(out=t[:], in_=b_in[name][:])
                bsb[name] = t
            emb_sb = cp_.tile([128, 2 * K], F8, tag="embp_sb")
            nc.sync.dma_start(out=emb_sb[:], in_=embp[:])
            e2_sb = cp_.tile([128, K], F32, tag="e2sb")
            nc.sync.dma_start(out=e2_sb[:], in_=e2bc[:])
            ident = cp_.tile([128, 128], F16, tag="ident")
            make_identity(nc, ident[:])

            inds_sb = cp_.tile([128, nt * NG], U32, tag="inds_sb")
            maxs_sb = cp_.tile([128, nt * NG], F32, tag="maxs_sb")
            musq_sb = cp_.tile([128, nt * 2], F32, tag="musq_sb")

            def dlayer(hpairs, wname, m_out, scale, bname, tprefix,
                       out_pairs=True, extra_epilogue=None):
                w_chunks = wsb[wname]
                nk = len(hpairs)
                outs = [ap_.tile([128, 2 * BT], F8, tag=f"{tprefix}_{i}",
                                 name=f"{tprefix}_{i}")
                        for i in range(m_out // 256)] if out_pairs else []
                for m, (mr0, mrows) in enumerate(_kchunks(m_out)):
                    p = ps.tile([128, BT], F32, tag="mm")
                    for ci in range(nk):
                        nc.tensor.matmul(
                            p[:mrows],
                            pair(w_chunks[ci][:])[:, :, mr0:mr0 + mrows],
                            pair(hpairs[ci][:]),
                            start=(ci == 0), stop=(ci == nk - 1),
                            perf_mode=DR,
                        )
                    if out_pairs:
                        o = outs[m // 2][:, (m % 2) * BT:(m % 2 + 1) * BT]
                        nc.scalar.activation(o, p[:mrows], AF.Relu,
                                             scale=scale,
                                             bias=bsb[bname][:mrows, m:m + 1])
                    if extra_epilogue is not None:
                        extra_epilogue(m, mrows, p)
                return outs

            def layer(h_chunks, h_ks, wname, bname, m_out, func, out_dt,
                      tprefix):
                outs = []
                w_chunks = wsb[wname]
                for m, (mr0, mrows) in enumerate(_kchunks(m_out)):
                    p = ps.tile([128, BT], F32, tag="mm")
                    nk = len(h_chunks)
                    for ki2 in range(nk):
                        nc.tensor.matmul(
                            p[:mrows],
                            w_chunks[ki2][:h_ks[ki2], mr0:mr0 + mrows],
                            h_chunks[ki2][:h_ks[ki2]],
                            start=(ki2 == 0), stop=(ki2 == nk - 1),
                        )
                    o = ap_.tile([128, BT], out_dt, tag=f"{tprefix}_{m}")
                    nc.scalar.activation(o[:mrows], p[:mrows], func,
                                         bias=bsb[bname][:mrows, m:m + 1])
                    outs.append(o)
                return outs

            hk = [128] * 4
            pend = None

            def decode(q_nats, cTt, t):
                b0 = t * BT
                qT = [ap_.tile([128, BT], F16, tag=f"qT_{lc}",
                               name=f"qT_{lc}_{t}") for lc in range(2)]
                for g in range(NG):
                    gsl = slice(g * 128, (g + 1) * 128)
                    for lc in range(2):
                        pT = psT.tile([128, 128], F16, tag="pT")
                        nc.tensor.transpose(
                            pT[:], q_nats[g][:, lc * 128:(lc + 1) * 128],
                            ident[:])
                        nc.scalar.activation(qT[lc][:, gsl], pT[:], AF.Copy)
                d_chunks = [qT[0], qT[1], cTt[0], cTt[1], cTt[2]]
                hd = layer(d_chunks, [128] * 5, "w4", "b4", H1, AF.Relu,
                           F16, "hd1")
                hd = layer(hd, hk, "w5", "b5", H1, AF.Relu, F16, "hd2")
                hd = layer(hd, hk, "w6", "b6", H1, AF.Relu, F16, "hd3")
                ro = layer(hd, hk, "wo", "bo", FRAME, AF.Identity, F32, "ro")
                for m, (mr0, mrows) in enumerate(_kchunks(FRAME)):
                    nc.sync.dma_start(out=reconT[mr0:mr0 + mrows, b0:b0 + BT],
                                      in_=ro[m][:mrows])

            for t in range(nt):
                b0 = t * BT
                sTt = []
                for ci in range(3):
                    tt = io.tile([128, 2 * BT], F8, tag=f"sT_{ci}",
                                 name=f"sT_{ci}_{t}")
                    nc.sync.dma_start(
                        out=tt[:, 0:BT],
                        in_=sT8[ci * 128:(ci + 1) * 128, b0:b0 + BT])
                    nc.sync.dma_start(
                        out=tt[:, BT:2 * BT],
                        in_=sT8[ci * 128:(ci + 1) * 128, bs + b0:bs + b0 + BT])
                    sTt.append(tt)
                cTt = []
                for ci in range(3):
                    tt = io.tile([128, BT], F16, tag=f"cT_{ci}",
                                 name=f"cT_{ci}_{t}")
                    nc.sync.dma_start(out=tt[:],
                                      in_=cT[ci * 128:(ci + 1) * 128,
                                             b0:b0 + BT])
                    cTt.append(tt)

                # encoder: L1 psum=1024*pre -> h1=32*relu; L2 psum=2048*pre
                # -> h2=64*relu; L3 psum=4096*pre -> h3=64*relu
                h = dlayer(sTt, "w1", H1, 1.0 / 32.0, "b1", "h1")
                h = dlayer(h, "w2", H1, 1.0 / 32.0, "b2", "h2")
                h = dlayer(h, "w3", H1, 1.0 / 64.0, "b3", "h3")

                mu8 = ap_.tile([128, 2 * BT], F8, tag="mu8", name=f"mu8_{t}")

                def mu_epi(m, mrows, p, t=t, mu8=mu8):
                    nc.scalar.activation(
                        mu8[:, m * BT:(m + 1) * BT], p[:mrows], AF.Identity,
                        scale=1.0 / 16.0, bias=bsb["bmu"][:mrows, m:m + 1])
                    scr = ap_.tile([128, BT], F32, tag="musq_scr")
                    nc.scalar.activation(
                        scr[:mrows], p[:mrows], AF.Square,
                        scale=1.0 / 4096.0,
                        bias=bsb["bmusq"][:mrows, m:m + 1],
                        accum_out=musq_sb[:mrows, 2 * t + m:2 * t + m + 1])

                dlayer(h, "wmu", LAT, None, None, "muu", out_pairs=False,
                       extra_epilogue=mu_epi)

                q_nats = []
                for g in range(NG):
                    col = t * NG + g
                    S_sb = io.tile([128, K], F32, tag="S_sb")
                    for half in range(2):
                        hsl = slice(half * 512, (half + 1) * 512)
                        pS = psS.tile([128, 512], F32, tag="S")
                        nc.tensor.matmul(
                            pS[:],
                            pair(mu8[:])[:, :, g * 128:(g + 1) * 128],
                            pair(emb_sb[:])[:, :, hsl],
                            start=True, stop=True, perf_mode=DR)
                        nc.vector.tensor_tensor(out=S_sb[:, hsl], in0=pS[:],
                                                in1=e2_sb[:, hsl],
                                                op=mybir.AluOpType.add)
                    mv = io.tile([128, 8], F32, tag="mv")
                    mi = io.tile([128, 8], U32, tag="mi")
                    nc.vector.max(mv[:], S_sb[:])
                    nc.vector.max_index(mi[:], mv[:], S_sb[:])
                    nc.vector.tensor_copy(inds_sb[:, col:col + 1], mi[:, 0:1])
                    nc.vector.tensor_copy(maxs_sb[:, col:col + 1], mv[:, 0:1])

                    q_nat = io.tile([128, LAT], F16, tag=f"q_nat_{g}",
                                    name=f"q_nat_{g}_{t}", bufs=2)
                    nc.gpsimd.indirect_dma_start(
                        out=q_nat[:], out_offset=None, in_=embT[:],
                        in_offset=bass.IndirectOffsetOnAxis(
                            ap=inds_sb[:, col:col + 1], axis=0),
                    )
                    q_nats.append(q_nat)

                if pend is not None:
                    decode(*pend)
                pend = (q_nats, cTt, t)

            decode(*pend)

            nc.sync.dma_start(out=inds_o[:], in_=inds_sb[:])
            nc.sync.dma_start(out=maxs_o[:], in_=maxs_sb[:])
            nc.sync.dma_start(out=musq_o[:], in_=musq_sb[:])
    nc.compile()
    return nc


def make_in_maps(x, c, W1, b1, W2, b2, W3, b3, Wmu, bmu,
                 W4, b4, W5, b5, W6, b6, Wo, bo, embed, bs=BS, ncores=NCORES):
    f16 = np.float16
    f32 = np.float32
    f8 = ml_dtypes.float8_e4m3

    def bias_cols(b, ncols, scale=1.0):
        out = np.zeros((128, ncols), f32)
        b = np.asarray(b, f32) * scale
        for ci in range(ncols):
            seg = b[128 * ci:128 * (ci + 1)]
            out[:len(seg), ci] = seg
        return out

    def pair8(w, rows_pad, scale):
        wp = np.zeros((rows_pad, w.shape[1]), f32)
        wp[:w.shape[0]] = np.asarray(w, f32) * scale
        npair = rows_pad // 256
        v = wp.reshape(npair, 2, 128, w.shape[1])
        v = v.transpose(0, 2, 1, 3).reshape(npair * 128, 2 * w.shape[1])
        return np.ascontiguousarray(v.astype(f8))

    embed = np.asarray(embed, f32)
    e2 = np.sum(embed.astype(np.float64) ** 2, axis=0).astype(f32)
    w4 = np.asarray(W4, f32)
    w4c = np.zeros((CP, H1), f32)
    w4c[:SIN - FRAME, :] = w4[LAT:, :]

    common = dict(
        w1=pair8(W1, SINP, S_W), w2=pair8(W2, H1, S_W), w3=pair8(W3, H1, S_W),
        wmu=pair8(Wmu, H1, S_W),
        w4q=w4[:LAT, :].astype(f16), w4c=w4c.astype(f16),
        w5=np.asarray(W5, f32).astype(f16), w6=np.asarray(W6, f32).astype(f16),
        wo=np.asarray(Wo, f32).astype(f16),
        b1=bias_cols(b1, 4, S_H1), b2=bias_cols(b2, 4, S_H),
        b3=bias_cols(b3, 4, S_H), bmu=bias_cols(bmu, 2, S_MU),
        bmusq=bias_cols(bmu, 2, 1.0),
        b4=bias_cols(b4, 4), b5=bias_cols(b5, 4), b6=bias_cols(b6, 4),
        bo=bias_cols(bo, 3),
        embp=pair8(embed, LAT, S_E),
        e2bc=np.ascontiguousarray(
            np.broadcast_to((-0.5 * S_MU * S_E * e2)[None, :], (128, K))),
        embT=np.ascontiguousarray(embed.T.astype(f16)),
    )
    x = np.asarray(x, f32)
    c = np.asarray(c, f32)
    in_maps = []
    for i in range(ncores):
        sl = slice(i * bs, (i + 1) * bs)
        sp = np.zeros((SINP, bs), f32)
        sp[:FRAME] = x[sl].T
        sp[FRAME:SIN] = c[sl].T
        v = (sp * S_S).reshape(3, 2, 128, bs).transpose(0, 2, 1, 3)
        m = dict(common)
        m["sT8"] = np.ascontiguousarray(
            v.reshape(SINP // 2, 2 * bs).astype(f8))
        cpd = np.zeros((CP, bs), f16)
        cpd[:FRAME] = c[sl].T.astype(f16)
        m["cT"] = cpd
        in_maps.append(m)
    return in_maps


def postprocess(results, bs=BS):
    recon_parts = []
    musq_total = 0.0
    maxs_total = 0.0
    counts = np.zeros(K, np.int64)
    for r in results:
        recon_parts.append(np.ascontiguousarray(r["reconT"].T))
        musq_total += r["musq"].astype(np.float64).sum()
        maxs_total += r["maxs"].astype(np.float64).sum() / (S_MU * S_E)
        idx = r["inds"].T.reshape(-1).astype(np.int64)
        counts += np.bincount(idx, minlength=K)
    recon = np.concatenate(recon_parts, axis=0).astype(np.float32)
    n = counts.sum()
    loss = np.float32((musq_total - 2.0 * maxs_total) / (n * LAT))
    p = (counts / n).astype(np.float32)
    perp = np.float32(np.exp(-np.sum(p * np.log(p + np.float32(1e-10)))))
    return recon, loss, perp


_NC_CACHE = {}


def kernel(x, c, W1, b1, W2, b2, W3, b3, Wmu, bmu,
           W4, b4, W5, b5, W6, b6, Wo, bo, embed, **run_kwargs):
    if BS not in _NC_CACHE:
        _NC_CACHE[BS] = build_nc(BS)
    nc = _NC_CACHE[BS]
    in_maps = make_in_maps(x, c, W1, b1, W2, b2, W3, b3, Wmu, bmu,
                           W4, b4, W5, b5, W6, b6, Wo, bo, embed)
    res = run_bass_kernel_spmd(nc, in_maps, list(range(NCORES)), **run_kwargs)
    out = postprocess(res.results)
    kernel.last_results = res
    return out


# revision 22
# speedup vs baseline: 1.0680x; 1.0536x over previous
"""PoseVQVAE forward — fp8(e4m3) DoubleRow encoder + VQ scoring, fp16 decoder.

Same structure as the fp16 kernel, but the encoder (L1..L3, Wmu) and the VQ
scoring matmul run in fp8-e4m3 with DoubleRow (2 contraction rows/cell, 2x
PE throughput). Power-of-2 scales keep every tensor comfortably inside e4m3
range (TRN e4m3 overflows to Inf at 256); fp32 PSUM accumulation. Validated
offline: VQ argmin decision margins are >=3.5 vs ~0.1 worst-case score
error, so the selected codebook rows — and therefore the decoder input and
recon — are unchanged vs the fp32 pipeline. Decoder stays fp16; c ships
separately in fp16 for it.
"""
import sys

sys.path.insert(0, "/opt/trn_rl_repo")

import ml_dtypes
import numpy as np

import concourse.bass as bass
import concourse.bacc as bacc
import concourse.mybir as mybir
import concourse.tile as tile
from concourse.bass_utils import run_bass_kernel_spmd
from concourse.masks import make_identity

F32 = mybir.dt.float32
F16 = mybir.dt.float16
F8 = mybir.dt.float8e4
U32 = mybir.dt.uint32
AF = mybir.ActivationFunctionType
DR = mybir.MatmulPerfMode.DoubleRow

B = 131072
NCORES = 8
BS = B // NCORES
FRAME = 267
SIN = 2 * FRAME
SINP = 768  # 3 x 256 DoubleRow pair chunks
CP = 384  # c rows padded to 3 x 128 for the decoder
H1 = 512
LAT = 256
K = 1024
BT = 512
NG = BT // 128

S_S, S_W, S_H1, S_H, S_MU, S_E = 16.0, 64.0, 32.0, 64.0, 256.0, 32.0


def _kchunks(n):
    out = []
    r = 0
    while r < n:
        out.append((r, min(128, n - r)))
        r += 128
    return out


def build_nc(bs=BS):
    nt = bs // BT
    nc = bacc.Bacc("TRN2", target_bir_lowering=False)

    sT8 = nc.dram_tensor("sT8", [SINP // 2, 2 * bs], F8, kind="ExternalInput")
    cT = nc.dram_tensor("cT", [CP, bs], F16, kind="ExternalInput")
    w_in = {}
    for name, shp, dt in (
        ("w1", [SINP // 2, 2 * H1], F8), ("w2", [H1 // 2, 2 * H1], F8),
        ("w3", [H1 // 2, 2 * H1], F8), ("wmu", [H1 // 2, 2 * LAT], F8),
        ("w4q", [LAT, H1], F16), ("w4c", [CP, H1], F16),
        ("w5", [H1, H1], F16), ("w6", [H1, H1], F16), ("wo", [H1, FRAME], F16),
    ):
        w_in[name] = nc.dram_tensor(name, shp, dt, kind="ExternalInput")
    b_in = {}
    for name, cols in (("b1", 4), ("b2", 4), ("b3", 4), ("bmu", 2),
                       ("bmusq", 2), ("b4", 4), ("b5", 4), ("b6", 4),
                       ("bo", 3)):
        b_in[name] = nc.dram_tensor(name, [128, cols], F32, kind="ExternalInput")
    embp = nc.dram_tensor("embp", [LAT // 2, 2 * K], F8, kind="ExternalInput")
    e2bc = nc.dram_tensor("e2bc", [128, K], F32, kind="ExternalInput")
    embT = nc.dram_tensor("embT", [K, LAT], F16, kind="ExternalInput")

    reconT = nc.dram_tensor("reconT", [FRAME, bs], F32, kind="ExternalOutput")
    inds_o = nc.dram_tensor("inds", [128, nt * NG], U32, kind="ExternalOutput")
    maxs_o = nc.dram_tensor("maxs", [128, nt * NG], F32, kind="ExternalOutput")
    musq_o = nc.dram_tensor("musq", [128, nt * 2], F32, kind="ExternalOutput")

    def pair(ap2d):
        return ap2d.rearrange("p (two n) -> p two n", two=2)

    with tile.TileContext(nc) as tc:
        with tc.tile_pool(name="consts", bufs=1) as cp_, \
             tc.tile_pool(name="io", bufs=4) as io, \
             tc.tile_pool(name="act", bufs=3) as ap_, \
             tc.tile_pool(name="ps", bufs=4, space="PSUM") as ps, \
             tc.tile_pool(name="psS", bufs=2, space="PSUM") as psS, \
             tc.tile_pool(name="psT", bufs=2, space="PSUM") as psT:

            wsb = {}
            for name in ("w1", "w2", "w3", "wmu", "w4q", "w4c", "w5", "w6",
                         "wo"):
                rows, cols = w_in[name].shape
                dt = w_in[name].dtype
                chunks = []
                for ci, (r0, rr) in enumerate(_kchunks(rows)):
                    t = cp_.tile([128, cols], dt, tag=f"{name}_{ci}")
                    nc.sync.dma_start(out=t[:rr], in_=w_in[name][r0:r0 + rr, :])
                    chunks.append(t)
                wsb[name] = chunks
            wsb["w4"] = [wsb["w4q"][0], wsb["w4q"][1],
                         wsb["w4c"][0], wsb["w4c"][1], wsb["w4c"][2]]
            bsb = {}
            for name in b_in:
                t = cp_.tile(list(b_in[name].shape), F32, tag=f"bs_{name}")
                nc.sync.dma_start(out=t[:], in_=b_in[name][:])
                bsb[name] = t
            emb_sb = cp_.tile([128, 2 * K], F8, tag="embp_sb")
            nc.sync.dma_start(out=emb_sb[:], in_=embp[:])
            e2_sb = cp_.tile([128, K], F32, tag="e2sb")
            nc.sync.dma_start(out=e2_sb[:], in_=e2bc[:])
            ident = cp_.tile([128, 128], F16, tag="ident")
            make_identity(nc, ident[:])

            inds_sb = cp_.tile([128, nt * NG], U32, tag="inds_sb")
            maxs_sb = cp_.tile([128, nt * NG], F32, tag="maxs_sb")
            musq_sb = cp_.tile([128, nt * 2], F32, tag="musq_sb")

            def dlayer(hpairs, wname, m_out, scale, bname, tprefix,
                       out_pairs=True, extra_epilogue=None):
                w_chunks = wsb[wname]
                nk = len(hpairs)
                outs = [ap_.tile([128, 2 * BT], F8, tag=f"{tprefix}_{i}",
                                 name=f"{tprefix}_{i}")
                        for i in range(m_out // 256)] if out_pairs else []
                for m, (mr0, mrows) in enumerate(_kchunks(m_out)):
                    p = ps.tile([128, BT], F32, tag="mm")
                    for ci in range(nk):
                        nc.tensor.matmul(
                            p[:mrows],
                            pair(w_chunks[ci][:])[:, :, mr0:mr0 + mrows],
                            pair(hpairs[ci][:]),
                            start=(ci == 0), stop=(ci == nk - 1),
                            perf_mode=DR,
                        )
                    if out_pairs:
                        o = outs[m // 2][:, (m % 2) * BT:(m % 2 + 1) * BT]
                        nc.scalar.activation(o, p[:mrows], AF.Relu,
                                             scale=scale,
                                             bias=bsb[bname][:mrows, m:m + 1])
                    if extra_epilogue is not None:
                        extra_epilogue(m, mrows, p)
                return outs

            def layer(h_chunks, h_ks, wname, bname, m_out, func, out_dt,
                      tprefix):
                outs = []
                w_chunks = wsb[wname]
                for m, (mr0, mrows) in enumerate(_kchunks(m_out)):
                    p = ps.tile([128, BT], F32, tag="mm")
                    nk = len(h_chunks)
                    for ki2 in range(nk):
                        nc.tensor.matmul(
                            p[:mrows],
                            w_chunks[ki2][:h_ks[ki2], mr0:mr0 + mrows],
                            h_chunks[ki2][:h_ks[ki2]],
                            start=(ki2 == 0), stop=(ki2 == nk - 1),
                        )
                    o = ap_.tile([128, BT], out_dt, tag=f"{tprefix}_{m}")
                    nc.scalar.activation(o[:mrows], p[:mrows], func,
                                         bias=bsb[bname][:mrows, m:m + 1])
                    outs.append(o)
                return outs

            hk = [128] * 4
            pend = None

            def decode(q_nats, cTt, t):
                b0 = t * BT
                qT = [ap_.tile([128, BT], F16, tag=f"qT_{lc}",
                               name=f"qT_{lc}_{t}") for lc in range(2)]
                for lc in range(2):
                    # all 4 group-transposes land in one f16 PSUM bank, then
                    # a single ACT copy delivers the whole qT tile
                    pT = psT.tile([128, BT], F16, tag="pT",
                                  name=f"pT_{lc}_{t}")
                    for g in range(NG):
                        nc.tensor.transpose(
                            pT[:, g * 128:(g + 1) * 128],
                            q_nats[g][:, lc * 128:(lc + 1) * 128],
                            ident[:])
                    nc.scalar.activation(qT[lc][:], pT[:], AF.Copy)
                d_chunks = [qT[0], qT[1], cTt[0], cTt[1], cTt[2]]
                hd = layer(d_chunks, [128] * 5, "w4", "b4", H1, AF.Relu,
                           F16, "hd1")
                hd = layer(hd, hk, "w5", "b5", H1, AF.Relu, F16, "hd2")
                hd = layer(hd, hk, "w6", "b6", H1, AF.Relu, F16, "hd3")
                ro = layer(hd, hk, "wo", "bo", FRAME, AF.Identity, F32, "ro")
                for m, (mr0, mrows) in enumerate(_kchunks(FRAME)):
                    nc.sync.dma_start(out=reconT[mr0:mr0 + mrows, b0:b0 + BT],
                                      in_=ro[m][:mrows])

            for t in range(nt):
                b0 = t * BT
                sTt = []
                for ci in range(3):
                    tt = io.tile([128, 2 * BT], F8, tag=f"sT_{ci}",
                                 name=f"sT_{ci}_{t}")
                    nc.sync.dma_start(
                        out=tt[:, 0:BT],
                        in_=sT8[ci * 128:(ci + 1) * 128, b0:b0 + BT])
                    nc.sync.dma_start(
                        out=tt[:, BT:2 * BT],
                        in_=sT8[ci * 128:(ci + 1) * 128, bs + b0:bs + b0 + BT])
                    sTt.append(tt)
                cTt = []
                for ci in range(3):
                    tt = io.tile([128, BT], F16, tag=f"cT_{ci}",
                                 name=f"cT_{ci}_{t}")
                    nc.sync.dma_start(out=tt[:],
                                      in_=cT[ci * 128:(ci + 1) * 128,
                                             b0:b0 + BT])
                    cTt.append(tt)

                # encoder: L1 psum=1024*pre -> h1=32*relu; L2 psum=2048*pre
                # -> h2=64*relu; L3 psum=4096*pre -> h3=64*relu
                h = dlayer(sTt, "w1", H1, 1.0 / 32.0, "b1", "h1")
                h = dlayer(h, "w2", H1, 1.0 / 32.0, "b2", "h2")
                h = dlayer(h, "w3", H1, 1.0 / 64.0, "b3", "h3")

                mu8 = ap_.tile([128, 2 * BT], F8, tag="mu8", name=f"mu8_{t}")

                def mu_epi(m, mrows, p, t=t, mu8=mu8):
                    nc.scalar.activation(
                        mu8[:, m * BT:(m + 1) * BT], p[:mrows], AF.Identity,
                        scale=1.0 / 16.0, bias=bsb["bmu"][:mrows, m:m + 1])
                    scr = ap_.tile([128, BT], F32, tag="musq_scr")
                    nc.scalar.activation(
                        scr[:mrows], p[:mrows], AF.Square,
                        scale=1.0 / 4096.0,
                        bias=bsb["bmusq"][:mrows, m:m + 1],
                        accum_out=musq_sb[:mrows, 2 * t + m:2 * t + m + 1])

                dlayer(h, "wmu", LAT, None, None, "muu", out_pairs=False,
                       extra_epilogue=mu_epi)

                q_nats = []
                for g in range(NG):
                    col = t * NG + g
                    S_sb = io.tile([128, K], F32, tag="S_sb")
                    for half in range(2):
                        hsl = slice(half * 512, (half + 1) * 512)
                        pS = psS.tile([128, 512], F32, tag="S")
                        nc.tensor.matmul(
                            pS[:],
                            pair(mu8[:])[:, :, g * 128:(g + 1) * 128],
                            pair(emb_sb[:])[:, :, hsl],
                            start=True, stop=True, perf_mode=DR)
                        nc.vector.tensor_tensor(out=S_sb[:, hsl], in0=pS[:],
                                                in1=e2_sb[:, hsl],
                                                op=mybir.AluOpType.add)
                    mv = io.tile([128, 8], F32, tag="mv")
                    mi = io.tile([128, 8], U32, tag="mi")
                    nc.vector.max(mv[:], S_sb[:])
                    nc.vector.max_index(mi[:], mv[:], S_sb[:])
                    nc.vector.tensor_copy(inds_sb[:, col:col + 1], mi[:, 0:1])
                    nc.vector.tensor_copy(maxs_sb[:, col:col + 1], mv[:, 0:1])

                    q_nat = io.tile([128, LAT], F16, tag=f"q_nat_{g}",
                                    name=f"q_nat_{g}_{t}", bufs=2)
                    nc.gpsimd.indirect_dma_start(
                        out=q_nat[:], out_offset=None, in_=embT[:],
                        in_offset=bass.IndirectOffsetOnAxis(
                            ap=inds_sb[:, col:col + 1], axis=0),
                    )
                    q_nats.append(q_nat)

                if pend is not None:
                    decode(*pend)
                pend = (q_nats, cTt, t)

            decode(*pend)

            nc.sync.dma_start(out=inds_o[:], in_=inds_sb[:])
            nc.sync.dma_start(out=maxs_o[:], in_=maxs_sb[:])
            nc.sync.dma_start(out=musq_o[:], in_=musq_sb[:])
    nc.compile()
    return nc


def make_in_maps(x, c, W1, b1, W2, b2, W3, b3, Wmu, bmu,
                 W4, b4, W5, b5, W6, b6, Wo, bo, embed, bs=BS, ncores=NCORES):
    f16 = np.float16
    f32 = np.float32
    f8 = ml_dtypes.float8_e4m3

    def bias_cols(b, ncols, scale=1.0):
        out = np.zeros((128, ncols), f32)
        b = np.asarray(b, f32) * scale
        for ci in range(ncols):
            seg = b[128 * ci:128 * (ci + 1)]
            out[:len(seg), ci] = seg
        return out

    def pair8(w, rows_pad, scale):
        wp = np.zeros((rows_pad, w.shape[1]), f32)
        wp[:w.shape[0]] = np.asarray(w, f32) * scale
        npair = rows_pad // 256
        v = wp.reshape(npair, 2, 128, w.shape[1])
        v = v.transpose(0, 2, 1, 3).reshape(npair * 128, 2 * w.shape[1])
        return np.ascontiguousarray(v.astype(f8))

    embed = np.asarray(embed, f32)
    e2 = np.sum(embed.astype(np.float64) ** 2, axis=0).astype(f32)
    w4 = np.asarray(W4, f32)
    w4c = np.zeros((CP, H1), f32)
    w4c[:SIN - FRAME, :] = w4[LAT:, :]

    common = dict(
        w1=pair8(W1, SINP, S_W), w2=pair8(W2, H1, S_W), w3=pair8(W3, H1, S_W),
        wmu=pair8(Wmu, H1, S_W),
        w4q=w4[:LAT, :].astype(f16), w4c=w4c.astype(f16),
        w5=np.asarray(W5, f32).astype(f16), w6=np.asarray(W6, f32).astype(f16),
        wo=np.asarray(Wo, f32).astype(f16),
        b1=bias_cols(b1, 4, S_H1), b2=bias_cols(b2, 4, S_H),
        b3=bias_cols(b3, 4, S_H), bmu=bias_cols(bmu, 2, S_MU),
        bmusq=bias_cols(bmu, 2, 1.0),
        b4=bias_cols(b4, 4), b5=bias_cols(b5, 4), b6=bias_cols(b6, 4),
        bo=bias_cols(bo, 3),
        embp=pair8(embed, LAT, S_E),
        e2bc=np.ascontiguousarray(
            np.broadcast_to((-0.5 * S_MU * S_E * e2)[None, :], (128, K))),
        embT=np.ascontiguousarray(embed.T.astype(f16)),
    )
    x = np.asarray(x, f32)
    c = np.asarray(c, f32)
    in_maps = []
    for i in range(ncores):
        sl = slice(i * bs, (i + 1) * bs)
        sp = np.zeros((SINP, bs), f32)
        sp[:FRAME] = x[sl].T
        sp[FRAME:SIN] = c[sl].T
        v = (sp * S_S).reshape(3, 2, 128, bs).transpose(0, 2, 1, 3)
        m = dict(common)
        m["sT8"] = np.ascontiguousarray(
            v.reshape(SINP // 2, 2 * bs).astype(f8))
        cpd = np.zeros((CP, bs), f16)
        cpd[:FRAME] = c[sl].T.astype(f16)
        m["cT"] = cpd
        in_maps.append(m)
    return in_maps


def postprocess(results, bs=BS):
    recon_parts = []
    musq_total = 0.0
    maxs_total = 0.0
    counts = np.zeros(K, np.int64)
    for r in results:
        recon_parts.append(np.ascontiguousarray(r["reconT"].T))
        musq_total += r["musq"].astype(np.float64).sum()
        maxs_total += r["maxs"].astype(np.float64).sum() / (S_MU * S_E)
        idx = r["inds"].T.reshape(-1).astype(np.int64)
        counts += np.bincount(idx, minlength=K)
    recon = np.concatenate(recon_parts, axis=0).astype(np.float32)
    n = counts.sum()
    loss = np.float32((musq_total - 2.0 * maxs_total) / (n * LAT))
    p = (counts / n).astype(np.float32)
    perp = np.float32(np.exp(-np.sum(p * np.log(p + np.float32(1e-10)))))
    return recon, loss, perp


_NC_CACHE = {}


def kernel(x, c, W1, b1, W2, b2, W3, b3, Wmu, bmu,
           W4, b4, W5, b5, W6, b6, Wo, bo, embed, **run_kwargs):
    if BS not in _NC_CACHE:
        _NC_CACHE[BS] = build_nc(BS)
    nc = _NC_CACHE[BS]
    in_maps = make_in_maps(x, c, W1, b1, W2, b2, W3, b3, Wmu, bmu,
                           W4, b4, W5, b5, W6, b6, Wo, bo, embed)
    res = run_bass_kernel_spmd(nc, in_maps, list(range(NCORES)), **run_kwargs)
    out = postprocess(res.results)
    kernel.last_results = res
    return out


# revision 23
# speedup vs baseline: 1.0917x; 1.0222x over previous
"""PoseVQVAE forward — fp8(e4m3) DoubleRow encoder + VQ scoring, fp16 decoder.

Same structure as the fp16 kernel, but the encoder (L1..L3, Wmu) and the VQ
scoring matmul run in fp8-e4m3 with DoubleRow (2 contraction rows/cell, 2x
PE throughput). Power-of-2 scales keep every tensor comfortably inside e4m3
range (TRN e4m3 overflows to Inf at 256); fp32 PSUM accumulation. Validated
offline: VQ argmin decision margins are >=3.5 vs ~0.1 worst-case score
error, so the selected codebook rows — and therefore the decoder input and
recon — are unchanged vs the fp32 pipeline. Decoder stays fp16; c ships
separately in fp16 for it.
"""
import sys

sys.path.insert(0, "/opt/trn_rl_repo")

import ml_dtypes
import numpy as np

import concourse.bass as bass
import concourse.bacc as bacc
import concourse.mybir as mybir
import concourse.tile as tile
from concourse.bass_utils import run_bass_kernel_spmd
from concourse.masks import make_identity

F32 = mybir.dt.float32
F16 = mybir.dt.float16
F8 = mybir.dt.float8e4
U32 = mybir.dt.uint32
AF = mybir.ActivationFunctionType
DR = mybir.MatmulPerfMode.DoubleRow

B = 131072
NCORES = 8
BS = B // NCORES
FRAME = 267
SIN = 2 * FRAME
SINP = 768  # 3 x 256 DoubleRow pair chunks
CP = 384  # c rows padded to 3 x 128 for the decoder
H1 = 512
LAT = 256
K = 1024
BT = 512
NG = BT // 128

S_S, S_W, S_H1, S_H, S_MU, S_E = 16.0, 64.0, 32.0, 64.0, 256.0, 32.0


def _kchunks(n):
    out = []
    r = 0
    while r < n:
        out.append((r, min(128, n - r)))
        r += 128
    return out


def build_nc(bs=BS):
    nt = bs // BT
    nc = bacc.Bacc("TRN2", target_bir_lowering=False)

    sT8 = nc.dram_tensor("sT8", [SINP // 2, 2 * bs], F8, kind="ExternalInput")
    cT = nc.dram_tensor("cT", [CP, bs], F16, kind="ExternalInput")
    w_in = {}
    for name, shp, dt in (
        ("w1", [SINP // 2, 2 * H1], F8), ("w2", [H1 // 2, 2 * H1], F8),
        ("w3", [H1 // 2, 2 * H1], F8), ("wmu", [H1 // 2, 2 * LAT], F8),
        ("w4q", [LAT, H1], F16), ("w4c", [CP, H1], F16),
        ("w5", [H1, H1], F16), ("w6", [H1, H1], F16), ("wo", [H1, FRAME], F16),
    ):
        w_in[name] = nc.dram_tensor(name, shp, dt, kind="ExternalInput")
    b_in = {}
    for name, cols in (("b1", 4), ("b2", 4), ("b3", 4), ("bmu", 2),
                       ("bmusq", 2), ("b4", 4), ("b5", 4), ("b6", 4),
                       ("bo", 3)):
        b_in[name] = nc.dram_tensor(name, [128, cols], F32, kind="ExternalInput")
    embp = nc.dram_tensor("embp", [LAT // 2, 2 * K], F8, kind="ExternalInput")
    e2bc = nc.dram_tensor("e2bc", [128, K], F32, kind="ExternalInput")
    embT = nc.dram_tensor("embT", [K, LAT], F16, kind="ExternalInput")

    reconT = nc.dram_tensor("reconT", [FRAME, bs], F32, kind="ExternalOutput")
    inds_o = nc.dram_tensor("inds", [128, nt * NG], U32, kind="ExternalOutput")
    maxs_o = nc.dram_tensor("maxs", [128, nt * NG], F32, kind="ExternalOutput")
    musq_o = nc.dram_tensor("musq", [128, nt * 2], F32, kind="ExternalOutput")

    def pair(ap2d):
        return ap2d.rearrange("p (two n) -> p two n", two=2)

    with tile.TileContext(nc) as tc:
        with tc.tile_pool(name="consts", bufs=1) as cp_, \
             tc.tile_pool(name="io", bufs=4) as io, \
             tc.tile_pool(name="act", bufs=3) as ap_, \
             tc.tile_pool(name="ps", bufs=4, space="PSUM") as ps, \
             tc.tile_pool(name="psS", bufs=2, space="PSUM") as psS, \
             tc.tile_pool(name="psT", bufs=2, space="PSUM") as psT:

            wsb = {}
            for name in ("w1", "w2", "w3", "wmu", "w4q", "w4c", "w5", "w6",
                         "wo"):
                rows, cols = w_in[name].shape
                dt = w_in[name].dtype
                chunks = []
                for ci, (r0, rr) in enumerate(_kchunks(rows)):
                    t = cp_.tile([128, cols], dt, tag=f"{name}_{ci}")
                    nc.sync.dma_start(out=t[:rr], in_=w_in[name][r0:r0 + rr, :])
                    chunks.append(t)
                wsb[name] = chunks
            wsb["w4"] = [wsb["w4q"][0], wsb["w4q"][1],
                         wsb["w4c"][0], wsb["w4c"][1], wsb["w4c"][2]]
            bsb = {}
            for name in b_in:
                t = cp_.tile(list(b_in[name].shape), F32, tag=f"bs_{name}")
                nc.sync.dma_start(out=t[:], in_=b_in[name][:])
                bsb[name] = t
            emb_sb = cp_.tile([128, 2 * K], F8, tag="embp_sb")
            nc.sync.dma_start(out=emb_sb[:], in_=embp[:])
            e2_sb = cp_.tile([128, K], F32, tag="e2sb")
            nc.sync.dma_start(out=e2_sb[:], in_=e2bc[:])
            ident = cp_.tile([128, 128], F16, tag="ident")
            make_identity(nc, ident[:])

            inds_sb = cp_.tile([128, nt * NG], U32, tag="inds_sb")
            maxs_sb = cp_.tile([128, nt * NG], F32, tag="maxs_sb")
            musq_sb = cp_.tile([128, nt * 2], F32, tag="musq_sb")

            def dlayer(hpairs, wname, m_out, scale, bname, tprefix,
                       out_pairs=True, extra_epilogue=None):
                w_chunks = wsb[wname]
                nk = len(hpairs)
                outs = [ap_.tile([128, 2 * BT], F8, tag=f"{tprefix}_{i}",
                                 name=f"{tprefix}_{i}")
                        for i in range(m_out // 256)] if out_pairs else []
                for m, (mr0, mrows) in enumerate(_kchunks(m_out)):
                    p = ps.tile([128, BT], F32, tag="mm")
                    for ci in range(nk):
                        nc.tensor.matmul(
                            p[:mrows],
                            pair(w_chunks[ci][:])[:, :, mr0:mr0 + mrows],
                            pair(hpairs[ci][:]),
                            start=(ci == 0), stop=(ci == nk - 1),
                            perf_mode=DR,
                        )
                    if out_pairs:
                        o = outs[m // 2][:, (m % 2) * BT:(m % 2 + 1) * BT]
                        nc.scalar.activation(o, p[:mrows], AF.Relu,
                                             scale=scale,
                                             bias=bsb[bname][:mrows, m:m + 1])
                    if extra_epilogue is not None:
                        extra_epilogue(m, mrows, p)
                return outs

            def layer(h_chunks, h_ks, wname, bname, m_out, func, out_dt,
                      tprefix):
                outs = []
                w_chunks = wsb[wname]
                for m, (mr0, mrows) in enumerate(_kchunks(m_out)):
                    p = ps.tile([128, BT], F32, tag="mm")
                    nk = len(h_chunks)
                    for ki2 in range(nk):
                        nc.tensor.matmul(
                            p[:mrows],
                            w_chunks[ki2][:h_ks[ki2], mr0:mr0 + mrows],
                            h_chunks[ki2][:h_ks[ki2]],
                            start=(ki2 == 0), stop=(ki2 == nk - 1),
                        )
                    o = ap_.tile([128, BT], out_dt, tag=f"{tprefix}_{m}")
                    nc.scalar.activation(o[:mrows], p[:mrows], func,
                                         bias=bsb[bname][:mrows, m:m + 1])
                    outs.append(o)
                return outs

            hk = [128] * 4
            pend = None

            def decode(q_nats, cTt, t):
                b0 = t * BT
                qT = [ap_.tile([128, BT], F16, tag=f"qT_{lc}",
                               name=f"qT_{lc}_{t}") for lc in range(2)]
                for lc in range(2):
                    # all 4 group-transposes land in one f16 PSUM bank, then
                    # a single ACT copy delivers the whole qT tile
                    pT = psT.tile([128, BT], F16, tag="pT",
                                  name=f"pT_{lc}_{t}")
                    for g in range(NG):
                        nc.tensor.transpose(
                            pT[:, g * 128:(g + 1) * 128],
                            q_nats[g][:, lc * 128:(lc + 1) * 128],
                            ident[:])
                    nc.scalar.activation(qT[lc][:], pT[:], AF.Copy)
                d_chunks = [qT[0], qT[1], cTt[0], cTt[1], cTt[2]]
                hd = layer(d_chunks, [128] * 5, "w4", "b4", H1, AF.Relu,
                           F16, "hd1")
                hd = layer(hd, hk, "w5", "b5", H1, AF.Relu, F16, "hd2")
                hd = layer(hd, hk, "w6", "b6", H1, AF.Relu, F16, "hd3")
                ro = layer(hd, hk, "wo", "bo", FRAME, AF.Identity, F32, "ro")
                for m, (mr0, mrows) in enumerate(_kchunks(FRAME)):
                    nc.sync.dma_start(out=reconT[mr0:mr0 + mrows, b0:b0 + BT],
                                      in_=ro[m][:mrows])

            for t in range(nt):
                b0 = t * BT
                sTt = []
                for ci in range(3):
                    tt = io.tile([128, 2 * BT], F8, tag=f"sT_{ci}",
                                 name=f"sT_{ci}_{t}")
                    nc.sync.dma_start(
                        out=tt[:, 0:BT],
                        in_=sT8[ci * 128:(ci + 1) * 128, b0:b0 + BT])
                    nc.sync.dma_start(
                        out=tt[:, BT:2 * BT],
                        in_=sT8[ci * 128:(ci + 1) * 128, bs + b0:bs + b0 + BT])
                    sTt.append(tt)
                cTt = []
                for ci in range(3):
                    tt = io.tile([128, BT], F16, tag=f"cT_{ci}",
                                 name=f"cT_{ci}_{t}")
                    nc.sync.dma_start(out=tt[:],
                                      in_=cT[ci * 128:(ci + 1) * 128,
                                             b0:b0 + BT])
                    cTt.append(tt)

                # encoder: L1 psum=1024*pre -> h1=32*relu; L2 psum=2048*pre
                # -> h2=64*relu; L3 psum=4096*pre -> h3=64*relu
                h = dlayer(sTt, "w1", H1, 1.0 / 32.0, "b1", "h1")
                h = dlayer(h, "w2", H1, 1.0 / 32.0, "b2", "h2")
                h = dlayer(h, "w3", H1, 1.0 / 64.0, "b3", "h3")

                mu8 = ap_.tile([128, 2 * BT], F8, tag="mu8", name=f"mu8_{t}")

                def mu_epi(m, mrows, p, t=t, mu8=mu8):
                    nc.scalar.activation(
                        mu8[:, m * BT:(m + 1) * BT], p[:mrows], AF.Identity,
                        scale=1.0 / 16.0, bias=bsb["bmu"][:mrows, m:m + 1])
                    scr = ap_.tile([128, BT], F32, tag="musq_scr")
                    nc.scalar.activation(
                        scr[:mrows], p[:mrows], AF.Square,
                        scale=1.0 / 4096.0,
                        bias=bsb["bmusq"][:mrows, m:m + 1],
                        accum_out=musq_sb[:mrows, 2 * t + m:2 * t + m + 1])

                dlayer(h, "wmu", LAT, None, None, "muu", out_pairs=False,
                       extra_epilogue=mu_epi)

                q_nats = []
                for g in range(NG):
                    col = t * NG + g
                    S_sb = io.tile([128, K], F32, tag="S_sb")
                    for half in range(2):
                        hsl = slice(half * 512, (half + 1) * 512)
                        pS = psS.tile([128, 512], F32, tag="S")
                        nc.tensor.matmul(
                            pS[:],
                            pair(mu8[:])[:, :, g * 128:(g + 1) * 128],
                            pair(emb_sb[:])[:, :, hsl],
                            start=True, stop=True, perf_mode=DR)
                        nc.vector.tensor_tensor(out=S_sb[:, hsl], in0=pS[:],
                                                in1=e2_sb[:, hsl],
                                                op=mybir.AluOpType.add)
                    mv = io.tile([128, 8], F32, tag="mv")
                    mi = io.tile([128, 8], U32, tag="mi")
                    nc.vector.max(mv[:], S_sb[:])
                    nc.vector.max_index(mi[:], mv[:], S_sb[:])
                    nc.vector.tensor_copy(inds_sb[:, col:col + 1], mi[:, 0:1])
                    nc.vector.tensor_copy(maxs_sb[:, col:col + 1], mv[:, 0:1])

                if pend is not None:
                    decode(*pend)

                # gathers AFTER the deferred decode: they share the dynamic
                # DMA queue with the previous tile's gathers, and emitting
                # them first would make the decode's transposes wait on this
                # tile's whole scoring chain via the queue semaphore.
                for g in range(NG):
                    col = t * NG + g
                    q_nat = io.tile([128, LAT], F16, tag=f"q_nat_{g}",
                                    name=f"q_nat_{g}_{t}", bufs=2)
                    nc.gpsimd.indirect_dma_start(
                        out=q_nat[:], out_offset=None, in_=embT[:],
                        in_offset=bass.IndirectOffsetOnAxis(
                            ap=inds_sb[:, col:col + 1], axis=0),
                    )
                    q_nats.append(q_nat)
                pend = (q_nats, cTt, t)

            decode(*pend)

            nc.sync.dma_start(out=inds_o[:], in_=inds_sb[:])
            nc.sync.dma_start(out=maxs_o[:], in_=maxs_sb[:])
            nc.sync.dma_start(out=musq_o[:], in_=musq_sb[:])
    nc.compile()
    return nc


def make_in_maps(x, c, W1, b1, W2, b2, W3, b3, Wmu, bmu,
                 W4, b4, W5, b5, W6, b6, Wo, bo, embed, bs=BS, ncores=NCORES):
    f16 = np.float16
    f32 = np.float32
    f8 = ml_dtypes.float8_e4m3

    def bias_cols(b, ncols, scale=1.0):
        out = np.zeros((128, ncols), f32)
        b = np.asarray(b, f32) * scale
        for ci in range(ncols):
            seg = b[128 * ci:128 * (ci + 1)]
            out[:len(seg), ci] = seg
        return out

    def pair8(w, rows_pad, scale):
        wp = np.zeros((rows_pad, w.shape[1]), f32)
        wp[:w.shape[0]] = np.asarray(w, f32) * scale
        npair = rows_pad // 256
        v = wp.reshape(npair, 2, 128, w.shape[1])
        v = v.transpose(0, 2, 1, 3).reshape(npair * 128, 2 * w.shape[1])
        return np.ascontiguousarray(v.astype(f8))

    embed = np.asarray(embed, f32)
    e2 = np.sum(embed.astype(np.float64) ** 2, axis=0).astype(f32)
    w4 = np.asarray(W4, f32)
    w4c = np.zeros((CP, H1), f32)
    w4c[:SIN - FRAME, :] = w4[LAT:, :]

    common = dict(
        w1=pair8(W1, SINP, S_W), w2=pair8(W2, H1, S_W), w3=pair8(W3, H1, S_W),
        wmu=pair8(Wmu, H1, S_W),
        w4q=w4[:LAT, :].astype(f16), w4c=w4c.astype(f16),
        w5=np.asarray(W5, f32).astype(f16), w6=np.asarray(W6, f32).astype(f16),
        wo=np.asarray(Wo, f32).astype(f16),
        b1=bias_cols(b1, 4, S_H1), b2=bias_cols(b2, 4, S_H),
        b3=bias_cols(b3, 4, S_H), bmu=bias_cols(bmu, 2, S_MU),
        bmusq=bias_cols(bmu, 2, 1.0),
        b4=bias_cols(b4, 4), b5=bias_cols(b5, 4), b6=bias_cols(b6, 4),
        bo=bias_cols(bo, 3),
        embp=pair8(embed, LAT, S_E),
        e2bc=np.ascontiguousarray(
            np.broadcast_to((-0.5 * S_MU * S_E * e2)[None, :], (128, K))),
        embT=np.ascontiguousarray(embed.T.astype(f16)),
    )
    x = np.asarray(x, f32)
    c = np.asarray(c, f32)
    in_maps = []
    for i in range(ncores):
        sl = slice(i * bs, (i + 1) * bs)
        sp = np.zeros((SINP, bs), f32)
        sp[:FRAME] = x[sl].T
        sp[FRAME:SIN] = c[sl].T
        v = (sp * S_S).reshape(3, 2, 128, bs).transpose(0, 2, 1, 3)
        m = dict(common)
        m["sT8"] = np.ascontiguousarray(
            v.reshape(SINP // 2, 2 * bs).astype(f8))
        cpd = np.zeros((CP, bs), f16)
        cpd[:FRAME] = c[sl].T.astype(f16)
        m["cT"] = cpd
        in_maps.append(m)
    return in_maps


def postprocess(results, bs=BS):
    recon_parts = []
    musq_total = 0.0
    maxs_total = 0.0
    counts = np.zeros(K, np.int64)
    for r in results:
        recon_parts.append(np.ascontiguousarray(r["reconT"].T))
        musq_total += r["musq"].astype(np.float64).sum()
        maxs_total += r["maxs"].astype(np.float64).sum() / (S_MU * S_E)
        idx = r["inds"].T.reshape(-1).astype(np.int64)
        counts += np.bincount(idx, minlength=K)
    recon = np.concatenate(recon_parts, axis=0).astype(np.float32)
    n = counts.sum()
    loss = np.float32((musq_total - 2.0 * maxs_total) / (n * LAT))
    p = (counts / n).astype(np.float32)
    perp = np.float32(np.exp(-np.sum(p * np.log(p + np.float32(1e-10)))))
    return recon, loss, perp


_NC_CACHE = {}


def kernel(x, c, W1, b1, W2, b2, W3, b3, Wmu, bmu,
           W4, b4, W5, b5, W6, b6, Wo, bo, embed, **run_kwargs):
    if BS not in _NC_CACHE:
        _NC_CACHE[BS] = build_nc(BS)
    nc = _NC_CACHE[BS]
    in_maps = make_in_maps(x, c, W1, b1, W2, b2, W3, b3, Wmu, bmu,
                           W4, b4, W5, b5, W6, b6, Wo, bo, embed)
    res = run_bass_kernel_spmd(nc, in_maps, list(range(NCORES)), **run_kwargs)
    out = postprocess(res.results)
    kernel.last_results = res
    return out
